# revision 17
# baseline (speedup 1.0000x reference)
"""Multi-head attention (B=16, N=1024, C=768, H=12) on 8 TRN2 NeuronCores.

Sharding: data-parallel over batch — each core runs the full attention block
for 2 of the 16 batch elements; weights are replicated, no collectives.

v3 design, driven by HW microbenchmarks (see memory/trn2-attention-kernel-
facts): bf16 matmuls stream ~2 cols/cycle (512-col MM ~= 92ns with the same
stationary, +75ns on stationary change); tile_position packs do NOT run
concurrently on this walrus; fp32 matmuls are self-loading but ~4x slower
streams; ScalarE exp is ~(N+444)/1.2 ns per instruction.

Structure per core (2 batches):
  A: QKV projections, loop-ordered so every stationary streams 2 matmuls
     (LDW amortized). V is token-major with a ones column per head slot
     ([v|1], 65 cols) so softmax denominators fall out of the AV matmul.
  B: per (head-pair, batch): 8 mc steps of [QK^T 2x512 per head (same kT
     stationary), exp N=1024 on ScalarE, AV M=65 into 4 accumulators
     (hi x nh)]. The normalization epilogue (reciprocal of the sums row,
     fp32 broadcast matmuls, in-place multiply) is DISTRIBUTED across the
     next block's mc steps so the in-order PE queue never stalls on it.
  C: output projection, each wproj chunk stationary streams 4 matmuls.

Container-specific: walrus accepts max ONE semaphore wait per instruction —
excess waits are hoisted onto injected EventSemaphore instructions in the
BIR JSON before compile.
"""

import json

import numpy as np
import ml_dtypes
from contextlib import ExitStack

import concourse.bass as bass
import concourse.tile as tile
import concourse.bass2jax as b2j
import concourse.bass_utils as bu
from concourse import mybir
from concourse.bass_utils import run_bass_kernel_spmd

N_CORES = 8

# ---------------------------------------------------------------------------
# walrus single-wait workaround
# ---------------------------------------------------------------------------
_MAX_WAITS = 1
_orig_compile = bu.compile_bir_kernel


def _split_waits(bir_json: bytes) -> bytes:
    d = json.loads(bir_json)
    for f in d.get("functions", []):
        for blk in f.get("blocks", []):
            new_insts = []
            for inst in blk.get("instructions", []):
                si = inst.get("sync_info")
                waits = si.get("on_wait", []) if si else []
                if len(waits) > _MAX_WAITS:
                    extra, keep = waits[:-_MAX_WAITS], waits[-_MAX_WAITS:]
                    for ci in range(0, len(extra), _MAX_WAITS):
                        new_insts.append({
                            "debug": inst.get("debug", 0),
                            "engine": inst["engine"],
                            "ins": [],
                            "name": f"{inst['name']}-wsplit{ci}",
                            "opcode": "EventSemaphore",
                            "outs": [],
                            "sync_info": {
                                "on_update": [],
                                "on_wait": extra[ci:ci + _MAX_WAITS],
                            },
                        })
                    si["on_wait"] = keep
                new_insts.append(inst)
            blk["instructions"] = new_insts
    return json.dumps(d).encode()


def _patched_compile(bir_json, tmpdir, neff_name="file.neff"):
    return _orig_compile(_split_waits(bir_json), tmpdir, neff_name=neff_name)


def _install_patch():
    bu.compile_bir_kernel = _patched_compile
    b2j.compile_bir_kernel = _patched_compile


F32 = mybir.dt.float32
BF16 = mybir.dt.bfloat16

DIM = 768
NH = 12
HD = 64
SCALE = HD ** -0.5
NB = 2
N = 1024
NT = NB * N
NCC = DIM // 128
NHP = NH // 2
SW = 65  # vp slot width per head ([v | 1])


def build_attention_nc(reps: int = 1):
    nc = bass.Bass("TRN2", target_bir_lowering=False, debug=False)
    xT = nc.declare_dram_parameter("xT", [DIM, NT], BF16, isOutput=False)
    wqkvT = nc.declare_dram_parameter("wqkvT", [DIM, 3 * DIM], BF16, isOutput=False)
    wprojT = nc.declare_dram_parameter("wprojT", [DIM, DIM], BF16, isOutput=False)
    bias = nc.declare_dram_parameter("bias", [DIM, 1], F32, isOutput=False)
    out = nc.declare_dram_parameter("out", [DIM, NT], F32, isOutput=True)

    with tile.TileContext(nc) as tc:
        for rep in range(reps):
            _emit(nc, tc, xT, wqkvT, wprojT, bias, out, rep)
    return nc


def _emit(nc, tc, xT, wqkvT, wprojT, bias, out, rep):
    R = f"r{rep}_"
    with ExitStack() as ctx:
        p_const = ctx.enter_context(tc.tile_pool(name=R + "const", bufs=1))
        p_w = ctx.enter_context(tc.tile_pool(name=R + "w", bufs=1))
        p_qk = ctx.enter_context(tc.tile_pool(name=R + "qk", bufs=1))
        p_vp = ctx.enter_context(tc.tile_pool(name=R + "vp", bufs=1))
        p_aT = ctx.enter_context(tc.tile_pool(name=R + "aT", bufs=1))
        p_e = ctx.enter_context(tc.tile_pool(name=R + "E", bufs=16))
        p_rs = ctx.enter_context(tc.tile_pool(name=R + "rs", bufs=1))
        p_bc = ctx.enter_context(tc.tile_pool(name=R + "bc", bufs=2))
        p_ob = ctx.enter_context(tc.tile_pool(name=R + "ob", bufs=2))

        # ---- constants / weights / inputs ----
        bias_sb = []
        for oc in range(NCC):
            tbs = p_const.tile([128, 1], F32, name=R + f"bias_sb{oc}")
            nc.sync.dma_start(tbs[:], bias[oc * 128:(oc + 1) * 128, :])
            bias_sb.append(tbs)
        ones_f32 = p_const.tile([128, 64], F32, name=R + "ones_f32")
        nc.vector.memset(ones_f32[:], 1.0)

        wp_t = []
        for hp in range(NHP):
            t = p_w.tile([128, DIM], BF16, name=R + f"wp{hp}")
            nc.sync.dma_start(t[:], wprojT[hp * 128:(hp + 1) * 128, :])
            wp_t.append(t)
        qT_t = [p_qk.tile([128, NT], BF16, name=R + f"qT{i}") for i in range(NHP)]
        kT_t = [p_qk.tile([128, NT], BF16, name=R + f"kT{i}") for i in range(NHP)]
        vp_t = [p_vp.tile([128, NH * SW], BF16, name=R + f"vp{i}")
                for i in range(NT // 128)]
        for t in vp_t:  # ones columns for the softmax denominators
            nc.vector.memset(
                t[:].rearrange("p (h e) -> p h e", e=SW)[:, :, HD:SW], 1.0)
        aT_t = {}

        # ---- phase A: projections (each stationary streams 2 matmuls) ----
        with tc.tile_pool(name=R + "psA", bufs=2, space="PSUM") as p_psA, \
             tc.tile_pool(name=R + "aw", bufs=1) as p_aw:
            wq_t = []
            for c in range(NCC):
                t = p_aw.tile([128, 3 * DIM], BF16, name=R + f"wq{c}")
                nc.sync.dma_start(t[:], wqkvT[c * 128:(c + 1) * 128, :])
                wq_t.append(t)
            xb = []
            for c in range(NCC):
                t = p_aw.tile([128, NT], BF16, name=R + f"xb{c}")
                nc.sync.dma_start(t[:], xT[c * 128:(c + 1) * 128, :])
                xb.append(t)
            for nn in range(NT // 128):
                ps = p_psA.tile([128, 1024], F32, tag=R + "psA",
                                name=R + f"psv{nn}")
                for c in range(NCC):
                    nc.tensor.matmul(
                        ps[:, 0:512],
                        xb[c][:, nn * 128:(nn + 1) * 128],
                        wq_t[c][:, 2 * DIM:2 * DIM + 512],
                        start=(c == 0), stop=(c == NCC - 1),
                    )
                    nc.tensor.matmul(
                        ps[:, 512:768],
                        xb[c][:, nn * 128:(nn + 1) * 128],
                        wq_t[c][:, 2 * DIM + 512:3 * DIM],
                        start=(c == 0), stop=(c == NCC - 1),
                    )
                nc.vector.tensor_copy(
                    vp_t[nn][:].rearrange("p (h e) -> p h e", e=SW)[:, :, 0:HD],
                    ps[:, 0:768].rearrange("p (h d) -> p h d", d=HD),
                )
            for hp in range(NHP):
                for which, dst in ((0, qT_t[hp]), (1, kT_t[hp])):
                    wcol = (which * NHP + hp) * 128
                    for half in range(2):
                        ps = p_psA.tile([128, 1024], F32, tag=R + "psA",
                                        name=R + f"psq{hp}_{which}_{half}")
                        for c in range(NCC):
                            for q in range(2):
                                nc.tensor.matmul(
                                    ps[:, q * 512:(q + 1) * 512],
                                    wq_t[c][:, wcol:wcol + 128],
                                    xb[c][:, half * 1024 + q * 512:
                                          half * 1024 + (q + 1) * 512],
                                    start=(c == 0), stop=(c == NCC - 1),
                                )
                        nc.vector.tensor_copy(
                            dst[:, half * 1024:(half + 1) * 1024], ps[:])

        # ---- phase B: attention, software-pipelined with staged epilogues ----
        LAG = 2
        with tc.tile_pool(name=R + "psS", bufs=4, space="PSUM") as p_psS, \
             tc.tile_pool(name=R + "pa", bufs=4, space="PSUM") as p_pa:

            def make_stages(at, pa, e_q, hp, b, boff):
                """Deferred work of this block, one stage per step of the
                NEXT block: lagged AVs, then recips+copies (frees pa), then
                broadcasts + in-place normalize."""
                rs = p_rs.tile([128, 2048], F32, tag=R + "rs",
                               name=R + f"rs{b}_{hp}")
                bcs = {}

                def do_av(mc):
                    first, last = (mc == 0), (mc == N // 128 - 1)
                    for hi in range(2):
                        vslot = vp_t[(boff + mc * 128) // 128][
                            :, (2 * hp + hi) * SW:(2 * hp + hi + 1) * SW]
                        for nh in range(2):
                            nc.tensor.matmul(
                                pa[2 * hi + nh][0:SW, :],
                                vslot,
                                e_q.pop((mc, hi, nh))[:],
                                start=first, stop=last,
                            )

                def s_av(mc):
                    return lambda: do_av(mc)

                def s_finish():
                    do_av(N // 128 - 1)
                    for j in (0, 2, 1, 3):  # nh0 slices first for psB0
                        nc.vector.reciprocal(
                            rs[64:65, j * 512:(j + 1) * 512], pa[j][64:65, :])
                        hi, nh = j // 2, j % 2
                        nc.vector.tensor_copy(
                            at[hi * 64:(hi + 1) * 64,
                               nh * 512:(nh + 1) * 512],
                            pa[j][0:64, :])

                def s_psb(nh):
                    def go():
                        psB = p_psS.tile([128, 512], F32, tag=R + "psS",
                                         name=R + f"psB{b}_{hp}_{nh}")
                        for hi in range(2):
                            j = 2 * hi + nh
                            nc.tensor.matmul(
                                psB[hi * 64:(hi + 1) * 64, :],
                                ones_f32[64:65, 0:64],
                                rs[64:65, j * 512:(j + 1) * 512],
                                start=True, stop=True,
                                tile_position=(64, hi * 64),
                            )
                        bc = p_bc.tile([128, 512], BF16, tag=R + "bc")
                        nc.vector.tensor_copy(bc[:], psB[:])
                        bcs[nh] = bc
                    return go

                def s_mul(nh):
                    def go():
                        nc.vector.tensor_mul(
                            at[:, nh * 512:(nh + 1) * 512],
                            at[:, nh * 512:(nh + 1) * 512], bcs[nh][:])
                    return go

                return [s_av(N // 128 - 2), s_finish,
                        s_psb(0), s_psb(1), s_mul(0), s_mul(1)]

            stages = []
            for hp in range(NHP):
                for b in range(NB):
                    boff = b * N
                    at = p_aT.tile([128, N], BF16, name=R + f"aT{b}_{hp}")
                    e_q = {}
                    pa = None
                    for mc in range(N // 128):
                        for hi in range(2):
                            pb_ = hi * 64
                            pss = []
                            for nh in range(2):
                                ps = p_psS.tile(
                                    [128, 512], F32, tag=R + "psS",
                                    name=R + f"psS{b}_{hp}_{mc}_{hi}_{nh}")
                                nc.tensor.matmul(
                                    ps[:],
                                    kT_t[hp][pb_:pb_ + 64,
                                             boff + mc * 128:
                                             boff + (mc + 1) * 128],
                                    qT_t[hp][pb_:pb_ + 64,
                                             boff + nh * 512:
                                             boff + (nh + 1) * 512],
                                    start=True, stop=True,
                                    tile_position=(pb_, 0),
                                )
                                pss.append(ps)
                            for nh in range(2):
                                e = p_e.tile([128, 512], BF16, tag=R + "E")
                                nc.scalar.activation(
                                    e[:], pss[nh][:],
                                    mybir.ActivationFunctionType.Exp,
                                    scale=SCALE,
                                )
                                e_q[(mc, hi, nh)] = e
                        if stages:
                            stages.pop(0)()
                        if mc == LAG:
                            pa = [p_pa.tile([128, 512], F32, tag=R + "pa",
                                            name=R + f"pa{b}_{hp}_{j}")
                                  for j in range(4)]
                        if mc >= LAG:
                            first = (mc - LAG == 0)
                            for hi in range(2):
                                vslot = vp_t[(boff + (mc - LAG) * 128) // 128][
                                    :, (2 * hp + hi) * SW:
                                    (2 * hp + hi + 1) * SW]
                                for nh in range(2):
                                    nc.tensor.matmul(
                                        pa[2 * hi + nh][0:SW, :],
                                        vslot,
                                        e_q.pop((mc - LAG, hi, nh))[:],
                                        start=first, stop=False,
                                    )
                    aT_t[(b, hp)] = at
                    stages = make_stages(at, pa, e_q, hp, b, boff)
            for s in stages:
                s()

            # ---- phase C: transposed output projection ----
            for oc in range(NCC):
                psC = [p_psS.tile([128, 512], F32, tag=R + "psS",
                                  name=R + f"psC{oc}_{u}") for u in range(4)]
                for cp in range(NHP):
                    lhs = wp_t[cp][:, oc * 128:(oc + 1) * 128]
                    for b in range(NB):
                        for nh in range(2):
                            nc.tensor.matmul(
                                psC[2 * b + nh][:],
                                lhs,
                                aT_t[(b, cp)][:, nh * 512:(nh + 1) * 512],
                                start=(cp == 0), stop=(cp == NHP - 1))
                for b in range(NB):
                    ob = p_ob.tile([128, N], F32, tag=R + "ob")
                    for nh in range(2):
                        nc.vector.tensor_scalar_add(
                            ob[:, nh * 512:(nh + 1) * 512],
                            psC[2 * b + nh][:], bias_sb[oc][:])
                    nc.sync.dma_start(
                        out[oc * 128:(oc + 1) * 128, b * N:(b + 1) * N], ob[:])


# ---------------------------------------------------------------------------
# host wrapper
# ---------------------------------------------------------------------------
_CACHE = {}


def _prep_in_maps(x, w_qkv, w_proj, b_proj):
    x = np.asarray(x, dtype=np.float32)
    wqkvT = np.ascontiguousarray(np.asarray(w_qkv, dtype=np.float32).T
                                 ).astype(ml_dtypes.bfloat16)
    wprojT = np.ascontiguousarray(np.asarray(w_proj, dtype=np.float32).T
                                  ).astype(ml_dtypes.bfloat16)
    bias = np.asarray(b_proj, dtype=np.float32).reshape(DIM, 1).copy()
    in_maps = []
    for c in range(N_CORES):
        xs = x[c * NB:(c + 1) * NB]                       # [2, 1024, 768]
        xT = np.ascontiguousarray(xs.transpose(2, 0, 1).reshape(DIM, NT))
        in_maps.append({
            "xT": xT.astype(ml_dtypes.bfloat16),
            "wqkvT": wqkvT,
            "wprojT": wprojT,
            "bias": bias,
        })
    return in_maps


def kernel(x, w_qkv, w_proj, b_proj):
    _install_patch()
    if "nc" not in _CACHE:
        _CACHE["nc"] = build_attention_nc(1)
    nc = _CACHE["nc"]
    in_maps = _prep_in_maps(x, w_qkv, w_proj, b_proj)
    res = run_bass_kernel_spmd(nc, in_maps, core_ids=list(range(N_CORES)))
    shards = []
    for c in range(N_CORES):
        oT = res.results[c]["out"]                        # [768, 2048]
        shards.append(oT.T.reshape(NB, N, DIM))
    return np.ascontiguousarray(
        np.concatenate(shards, axis=0)).astype(np.float32)


# revision 19
# speedup vs baseline: 1.0315x; 1.0315x over previous
"""Multi-head attention (B=16, N=1024, C=768, H=12) on 8 TRN2 NeuronCores.

Sharding: data-parallel over batch — each core runs the full attention block
for 2 of the 16 batch elements; weights are replicated, no collectives.

v3 design, driven by HW microbenchmarks (see memory/trn2-attention-kernel-
facts): bf16 matmuls stream ~2 cols/cycle (512-col MM ~= 92ns with the same
stationary, +75ns on stationary change); tile_position packs do NOT run
concurrently on this walrus; fp32 matmuls are self-loading but ~4x slower
streams; ScalarE exp is ~(N+444)/1.2 ns per instruction.

Structure per core (2 batches):
  A: QKV projections, loop-ordered so every stationary streams 2 matmuls
     (LDW amortized). V is token-major with a ones column per head slot
     ([v|1], 65 cols) so softmax denominators fall out of the AV matmul.
  B: per (head-pair, batch): 8 mc steps of [QK^T 2x512 per head (same kT
     stationary), exp N=1024 on ScalarE, AV M=65 into 4 accumulators
     (hi x nh)]. The normalization epilogue (reciprocal of the sums row,
     fp32 broadcast matmuls, in-place multiply) is DISTRIBUTED across the
     next block's mc steps so the in-order PE queue never stalls on it.
  C: output projection, each wproj chunk stationary streams 4 matmuls.

Container-specific: walrus accepts max ONE semaphore wait per instruction —
excess waits are hoisted onto injected EventSemaphore instructions in the
BIR JSON before compile.
"""

import json

import numpy as np
import ml_dtypes
from contextlib import ExitStack

import concourse.bass as bass
import concourse.tile as tile
import concourse.bass2jax as b2j
import concourse.bass_utils as bu
from concourse import mybir
from concourse.bass_utils import run_bass_kernel_spmd

N_CORES = 8

# ---------------------------------------------------------------------------
# walrus single-wait workaround
# ---------------------------------------------------------------------------
_MAX_WAITS = 1
_orig_compile = bu.compile_bir_kernel


def _split_waits(bir_json: bytes) -> bytes:
    d = json.loads(bir_json)
    for f in d.get("functions", []):
        for blk in f.get("blocks", []):
            new_insts = []
            for inst in blk.get("instructions", []):
                si = inst.get("sync_info")
                waits = si.get("on_wait", []) if si else []
                if len(waits) > _MAX_WAITS:
                    extra, keep = waits[:-_MAX_WAITS], waits[-_MAX_WAITS:]
                    for ci in range(0, len(extra), _MAX_WAITS):
                        new_insts.append({
                            "debug": inst.get("debug", 0),
                            "engine": inst["engine"],
                            "ins": [],
                            "name": f"{inst['name']}-wsplit{ci}",
                            "opcode": "EventSemaphore",
                            "outs": [],
                            "sync_info": {
                                "on_update": [],
                                "on_wait": extra[ci:ci + _MAX_WAITS],
                            },
                        })
                    si["on_wait"] = keep
                new_insts.append(inst)
            blk["instructions"] = new_insts
    return json.dumps(d).encode()


def _patched_compile(bir_json, tmpdir, neff_name="file.neff"):
    return _orig_compile(_split_waits(bir_json), tmpdir, neff_name=neff_name)


def _install_patch():
    bu.compile_bir_kernel = _patched_compile
    b2j.compile_bir_kernel = _patched_compile


F32 = mybir.dt.float32
BF16 = mybir.dt.bfloat16

DIM = 768
NH = 12
HD = 64
SCALE = HD ** -0.5
NB = 2
N = 1024
NT = NB * N
NCC = DIM // 128
NHP = NH // 2
SW = 65  # vp slot width per head ([v | 1])


def build_attention_nc(reps: int = 1):
    nc = bass.Bass("TRN2", target_bir_lowering=False, debug=False)
    xT = nc.declare_dram_parameter("xT", [DIM, NT], BF16, isOutput=False)
    wqkvT = nc.declare_dram_parameter("wqkvT", [DIM, 3 * DIM], BF16, isOutput=False)
    wprojT = nc.declare_dram_parameter("wprojT", [DIM, DIM], BF16, isOutput=False)
    bias = nc.declare_dram_parameter("bias", [DIM, 1], F32, isOutput=False)
    out = nc.declare_dram_parameter("out", [DIM, NT], F32, isOutput=True)

    with tile.TileContext(nc) as tc:
        for rep in range(reps):
            _emit(nc, tc, xT, wqkvT, wprojT, bias, out, rep)
    return nc


def _emit(nc, tc, xT, wqkvT, wprojT, bias, out, rep):
    R = f"r{rep}_"
    with ExitStack() as ctx:
        p_const = ctx.enter_context(tc.tile_pool(name=R + "const", bufs=1))
        p_w = ctx.enter_context(tc.tile_pool(name=R + "w", bufs=1))
        p_qk = ctx.enter_context(tc.tile_pool(name=R + "qk", bufs=1))
        p_vp = ctx.enter_context(tc.tile_pool(name=R + "vp", bufs=1))
        p_aT = ctx.enter_context(tc.tile_pool(name=R + "aT", bufs=1))
        p_e = ctx.enter_context(tc.tile_pool(name=R + "E", bufs=16))
        p_rs = ctx.enter_context(tc.tile_pool(name=R + "rs", bufs=2))
        p_bc = ctx.enter_context(tc.tile_pool(name=R + "bc", bufs=2))
        p_ob = ctx.enter_context(tc.tile_pool(name=R + "ob", bufs=2))

        # ---- constants / weights / inputs ----
        bias_sb = []
        for oc in range(NCC):
            tbs = p_const.tile([128, 1], F32, name=R + f"bias_sb{oc}")
            nc.sync.dma_start(tbs[:], bias[oc * 128:(oc + 1) * 128, :])
            bias_sb.append(tbs)
        ones_bf = p_const.tile([128, 64], BF16, name=R + "ones_bf")
        nc.vector.memset(ones_bf[:], 1.0)

        wp_t = []
        for hp in range(NHP):
            t = p_w.tile([128, DIM], BF16, name=R + f"wp{hp}")
            nc.sync.dma_start(t[:], wprojT[hp * 128:(hp + 1) * 128, :])
            wp_t.append(t)
        qT_t = [p_qk.tile([128, NT], BF16, name=R + f"qT{i}") for i in range(NHP)]
        kT_t = [p_qk.tile([128, NT], BF16, name=R + f"kT{i}") for i in range(NHP)]
        vp_t = [p_vp.tile([128, NH * SW], BF16, name=R + f"vp{i}")
                for i in range(NT // 128)]
        for t in vp_t:  # ones columns for the softmax denominators
            nc.vector.memset(
                t[:].rearrange("p (h e) -> p h e", e=SW)[:, :, HD:SW], 1.0)
        aT_t = {}

        # ---- phase A: projections (each stationary streams 2 matmuls) ----
        with tc.tile_pool(name=R + "psA", bufs=2, space="PSUM") as p_psA, \
             tc.tile_pool(name=R + "aw", bufs=1) as p_aw:
            wq_t = []
            for c in range(NCC):
                t = p_aw.tile([128, 3 * DIM], BF16, name=R + f"wq{c}")
                nc.sync.dma_start(t[:], wqkvT[c * 128:(c + 1) * 128, :])
                wq_t.append(t)
            xb = []
            for c in range(NCC):
                t = p_aw.tile([128, NT], BF16, name=R + f"xb{c}")
                nc.sync.dma_start(t[:], xT[c * 128:(c + 1) * 128, :])
                xb.append(t)
            for nn in range(NT // 128):
                ps = p_psA.tile([128, 1024], F32, tag=R + "psA",
                                name=R + f"psv{nn}")
                for c in range(NCC):
                    nc.tensor.matmul(
                        ps[:, 0:512],
                        xb[c][:, nn * 128:(nn + 1) * 128],
                        wq_t[c][:, 2 * DIM:2 * DIM + 512],
                        start=(c == 0), stop=(c == NCC - 1),
                    )
                    nc.tensor.matmul(
                        ps[:, 512:768],
                        xb[c][:, nn * 128:(nn + 1) * 128],
                        wq_t[c][:, 2 * DIM + 512:3 * DIM],
                        start=(c == 0), stop=(c == NCC - 1),
                    )
                nc.vector.tensor_copy(
                    vp_t[nn][:].rearrange("p (h e) -> p h e", e=SW)[:, :, 0:HD],
                    ps[:, 0:768].rearrange("p (h d) -> p h d", d=HD),
                )
            for hp in range(NHP):
                for which, dst in ((0, qT_t[hp]), (1, kT_t[hp])):
                    wcol = (which * NHP + hp) * 128
                    for half in range(2):
                        ps = p_psA.tile([128, 1024], F32, tag=R + "psA",
                                        name=R + f"psq{hp}_{which}_{half}")
                        for c in range(NCC):
                            for q in range(2):
                                nc.tensor.matmul(
                                    ps[:, q * 512:(q + 1) * 512],
                                    wq_t[c][:, wcol:wcol + 128],
                                    xb[c][:, half * 1024 + q * 512:
                                          half * 1024 + (q + 1) * 512],
                                    start=(c == 0), stop=(c == NCC - 1),
                                )
                        nc.vector.tensor_copy(
                            dst[:, half * 1024:(half + 1) * 1024], ps[:])

        # ---- phase B: attention, software-pipelined with staged epilogues ----
        LAG = 2
        with tc.tile_pool(name=R + "psS", bufs=4, space="PSUM") as p_psS, \
             tc.tile_pool(name=R + "pa", bufs=4, space="PSUM") as p_pa:

            def make_stages(at, pa, e_q, hp, b, boff):
                """Deferred work of this block, one stage per step of the
                NEXT block: lagged AVs; denominator+output copies (free pa);
                partition-gather of the 4 sum rows; ONE wide reciprocal;
                bf16 broadcasts; in-place normalize."""
                drow = p_rs.tile([128, 2048], BF16, tag=R + "drow",
                                 name=R + f"drow{b}_{hp}")
                st = {}
                P_OF = {0: 0, 1: 32, 2: 64, 3: 96}

                def do_av(mc):
                    first, last = (mc == 0), (mc == N // 128 - 1)
                    for hi in range(2):
                        vslot = vp_t[(boff + mc * 128) // 128][
                            :, (2 * hp + hi) * SW:(2 * hp + hi + 1) * SW]
                        for nh in range(2):
                            nc.tensor.matmul(
                                pa[2 * hi + nh][0:SW, :],
                                vslot,
                                e_q.pop((mc, hi, nh))[:],
                                start=first, stop=last,
                            )

                def s0():
                    do_av(N // 128 - 2)

                def s1():
                    do_av(N // 128 - 1)
                    for j in range(4):  # denominator rows -> drow (bf16)
                        nc.vector.tensor_copy(
                            drow[64:65, j * 512:(j + 1) * 512],
                            pa[j][64:65, :])
                    for j in (0, 1):
                        nc.vector.tensor_copy(
                            at[0:64, (j % 2) * 512:(j % 2 + 1) * 512],
                            pa[j][0:64, :])

                def s2():
                    for j in (2, 3):
                        nc.vector.tensor_copy(
                            at[64:128, (j % 2) * 512:(j % 2 + 1) * 512],
                            pa[j][0:64, :])
                    psG = p_psS.tile([128, 512], F32, tag=R + "psS",
                                     name=R + f"psG{b}_{hp}")
                    nc.vector.memset(psG[:], 1.0)
                    for j in range(4):  # gather sums to partitions 0/32/64/96
                        nc.tensor.matmul(
                            psG[P_OF[j]:P_OF[j] + 1, :],
                            ones_bf[64:65, 0:1],
                            drow[64:65, j * 512:(j + 1) * 512],
                            start=True, stop=True,
                            tile_position=(64, P_OF[j]),
                            skip_group_check=True,
                        )
                    st["psG"] = psG

                def s3():
                    rs2 = p_rs.tile([128, 512], F32, tag=R + "rs2",
                                    name=R + f"rs2{b}_{hp}")
                    nc.vector.reciprocal(rs2[:], st["psG"][:])
                    rsb = p_rs.tile([128, 512], BF16, tag=R + "rsb",
                                    name=R + f"rsb{b}_{hp}")
                    nc.vector.tensor_copy(rsb[:], rs2[:])
                    st["rsb"] = rsb

                def s4():
                    for nh in range(2):
                        psB = p_psS.tile([128, 512], F32, tag=R + "psS",
                                         name=R + f"psB{b}_{hp}_{nh}")
                        for hi in range(2):
                            p = P_OF[2 * hi + nh]
                            nc.tensor.matmul(
                                psB[hi * 64:(hi + 1) * 64, :],
                                ones_bf[p:p + 1, 0:64],
                                st["rsb"][p:p + 1, :],
                                start=True, stop=True,
                                tile_position=(p, hi * 64),
                                skip_group_check=True,
                            )
                        bc = p_bc.tile([128, 512], BF16, tag=R + "bc")
                        nc.vector.tensor_copy(bc[:], psB[:])
                        st[f"bc{nh}"] = bc

                def s5():
                    for nh in range(2):
                        nc.vector.tensor_mul(
                            at[:, nh * 512:(nh + 1) * 512],
                            at[:, nh * 512:(nh + 1) * 512],
                            st[f"bc{nh}"][:])

                return [s0, s1, s2, s3, s4, s5]

            stages = []
            for hp in range(NHP):
                for b in range(NB):
                    boff = b * N
                    at = p_aT.tile([128, N], BF16, name=R + f"aT{b}_{hp}")
                    e_q = {}
                    pa = None
                    for mc in range(N // 128):
                        for hi in range(2):
                            pb_ = hi * 64
                            pss = []
                            for nh in range(2):
                                ps = p_psS.tile(
                                    [128, 512], F32, tag=R + "psS",
                                    name=R + f"psS{b}_{hp}_{mc}_{hi}_{nh}")
                                nc.tensor.matmul(
                                    ps[:],
                                    kT_t[hp][pb_:pb_ + 64,
                                             boff + mc * 128:
                                             boff + (mc + 1) * 128],
                                    qT_t[hp][pb_:pb_ + 64,
                                             boff + nh * 512:
                                             boff + (nh + 1) * 512],
                                    start=True, stop=True,
                                    tile_position=(pb_, 0),
                                )
                                pss.append(ps)
                            for nh in range(2):
                                e = p_e.tile([128, 512], BF16, tag=R + "E")
                                nc.scalar.activation(
                                    e[:], pss[nh][:],
                                    mybir.ActivationFunctionType.Exp,
                                    scale=SCALE,
                                )
                                e_q[(mc, hi, nh)] = e
                        if stages:
                            stages.pop(0)()
                        if mc == LAG:
                            pa = [p_pa.tile([128, 512], F32, tag=R + "pa",
                                            name=R + f"pa{b}_{hp}_{j}")
                                  for j in range(4)]
                        if mc >= LAG:
                            first = (mc - LAG == 0)
                            for hi in range(2):
                                vslot = vp_t[(boff + (mc - LAG) * 128) // 128][
                                    :, (2 * hp + hi) * SW:
                                    (2 * hp + hi + 1) * SW]
                                for nh in range(2):
                                    nc.tensor.matmul(
                                        pa[2 * hi + nh][0:SW, :],
                                        vslot,
                                        e_q.pop((mc - LAG, hi, nh))[:],
                                        start=first, stop=False,
                                    )
                    aT_t[(b, hp)] = at
                    stages = make_stages(at, pa, e_q, hp, b, boff)
            for s in stages:
                s()

            # ---- phase C: transposed output projection ----
            for oc in range(NCC):
                psC = [p_psS.tile([128, 512], F32, tag=R + "psS",
                                  name=R + f"psC{oc}_{u}") for u in range(4)]
                for cp in range(NHP):
                    lhs = wp_t[cp][:, oc * 128:(oc + 1) * 128]
                    for b in range(NB):
                        for nh in range(2):
                            nc.tensor.matmul(
                                psC[2 * b + nh][:],
                                lhs,
                                aT_t[(b, cp)][:, nh * 512:(nh + 1) * 512],
                                start=(cp == 0), stop=(cp == NHP - 1))
                for b in range(NB):
                    ob = p_ob.tile([128, N], F32, tag=R + "ob")
                    for nh in range(2):
                        nc.vector.tensor_scalar_add(
                            ob[:, nh * 512:(nh + 1) * 512],
                            psC[2 * b + nh][:], bias_sb[oc][:])
                    nc.sync.dma_start(
                        out[oc * 128:(oc + 1) * 128, b * N:(b + 1) * N], ob[:])


# ---------------------------------------------------------------------------
# host wrapper
# ---------------------------------------------------------------------------
_CACHE = {}


def _prep_in_maps(x, w_qkv, w_proj, b_proj):
    x = np.asarray(x, dtype=np.float32)
    wqkvT = np.ascontiguousarray(np.asarray(w_qkv, dtype=np.float32).T
                                 ).astype(ml_dtypes.bfloat16)
    wprojT = np.ascontiguousarray(np.asarray(w_proj, dtype=np.float32).T
                                  ).astype(ml_dtypes.bfloat16)
    bias = np.asarray(b_proj, dtype=np.float32).reshape(DIM, 1).copy()
    in_maps = []
    for c in range(N_CORES):
        xs = x[c * NB:(c + 1) * NB]                       # [2, 1024, 768]
        xT = np.ascontiguousarray(xs.transpose(2, 0, 1).reshape(DIM, NT))
        in_maps.append({
            "xT": xT.astype(ml_dtypes.bfloat16),
            "wqkvT": wqkvT,
            "wprojT": wprojT,
            "bias": bias,
        })
    return in_maps


def kernel(x, w_qkv, w_proj, b_proj):
    _install_patch()
    if "nc" not in _CACHE:
        _CACHE["nc"] = build_attention_nc(1)
    nc = _CACHE["nc"]
    in_maps = _prep_in_maps(x, w_qkv, w_proj, b_proj)
    res = run_bass_kernel_spmd(nc, in_maps, core_ids=list(range(N_CORES)))
    shards = []
    for c in range(N_CORES):
        oT = res.results[c]["out"]                        # [768, 2048]
        shards.append(oT.T.reshape(NB, N, DIM))
    return np.ascontiguousarray(
        np.concatenate(shards, axis=0)).astype(np.float32)


# revision 23
# speedup vs baseline: 1.1752x; 1.1393x over previous
"""Multi-head attention (B=16, N=1024, C=768, H=12) on 8 TRN2 NeuronCores.

Sharding: data-parallel over batch — each core runs the full attention block
for 2 of the 16 batch elements; weights are replicated, no collectives.

v3 design, driven by HW microbenchmarks (see memory/trn2-attention-kernel-
facts): bf16 matmuls stream ~2 cols/cycle (512-col MM ~= 92ns with the same
stationary, +75ns on stationary change); tile_position packs do NOT run
concurrently on this walrus; fp32 matmuls are self-loading but ~4x slower
streams; ScalarE exp is ~(N+444)/1.2 ns per instruction.

Structure per core (2 batches):
  A: QKV projections, loop-ordered so every stationary streams 2 matmuls
     (LDW amortized). V is token-major with a ones column per head slot
     ([v|1], 65 cols) so softmax denominators fall out of the AV matmul.
  B: per (head-pair, batch): 8 mc steps of [QK^T 2x512 per head (same kT
     stationary), exp N=1024 on ScalarE, AV M=65 into 4 accumulators
     (hi x nh)]. The normalization epilogue (reciprocal of the sums row,
     fp32 broadcast matmuls, in-place multiply) is DISTRIBUTED across the
     next block's mc steps so the in-order PE queue never stalls on it.
  C: output projection, each wproj chunk stationary streams 4 matmuls.

Container-specific: walrus accepts max ONE semaphore wait per instruction —
excess waits are hoisted onto injected EventSemaphore instructions in the
BIR JSON before compile.
"""

import json

import numpy as np
import ml_dtypes
from contextlib import ExitStack

import concourse.bass as bass
import concourse.tile as tile
import concourse.bass2jax as b2j
import concourse.bass_utils as bu
from concourse import mybir
from concourse.bass_utils import run_bass_kernel_spmd

N_CORES = 8
_ABLATE = set()

# ---------------------------------------------------------------------------
# walrus single-wait workaround
# ---------------------------------------------------------------------------
_MAX_WAITS = 1
_orig_compile = bu.compile_bir_kernel


def _split_waits(bir_json: bytes) -> bytes:
    d = json.loads(bir_json)
    for f in d.get("functions", []):
        for blk in f.get("blocks", []):
            new_insts = []
            for inst in blk.get("instructions", []):
                si = inst.get("sync_info")
                waits = si.get("on_wait", []) if si else []
                if len(waits) > _MAX_WAITS:
                    extra, keep = waits[:-_MAX_WAITS], waits[-_MAX_WAITS:]
                    for ci in range(0, len(extra), _MAX_WAITS):
                        new_insts.append({
                            "debug": inst.get("debug", 0),
                            "engine": inst["engine"],
                            "ins": [],
                            "name": f"{inst['name']}-wsplit{ci}",
                            "opcode": "EventSemaphore",
                            "outs": [],
                            "sync_info": {
                                "on_update": [],
                                "on_wait": extra[ci:ci + _MAX_WAITS],
                            },
                        })
                    si["on_wait"] = keep
                new_insts.append(inst)
            blk["instructions"] = new_insts
    return json.dumps(d).encode()


def _strip_self_waits(bir_json: bytes) -> bytes:
    """Remove waits that are provably satisfied by the waiting engine's own
    program order: a wait on semaphore S by an instruction on engine E is
    redundant if S is only ever incremented by earlier instructions on E and
    their cumulative increments already reach the wait value."""
    d = json.loads(bir_json)
    for f in d.get("functions", []):
        for blk in f.get("blocks", []):
            insts = blk.get("instructions", [])
            sem_updaters = {}
            for inst in insts:
                si = inst.get("sync_info") or {}
                for u in si.get("on_update", []):
                    eng = inst["engine"]
                    if "DMA" in inst.get("opcode", ""):
                        eng = "__async__"  # DMA updates fire out of order
                    sem_updaters.setdefault(u["id"], set()).add(eng)
            cum = {}
            for inst in insts:
                eng = inst["engine"]
                si = inst.get("sync_info")
                if si and si.get("on_wait"):
                    kept = []
                    for w in si["on_wait"]:
                        sid = w.get("id")
                        if (w.get("wait_mode") == "sem-ge-imm"
                                and sem_updaters.get(sid) == {eng}
                                and cum.get((eng, sid), 0) >= w["wait_value"]):
                            continue
                        kept.append(w)
                    si["on_wait"] = kept
                if si:
                    for u in si.get("on_update", []):
                        if u.get("update_mode") == "sem-inc":
                            key = (eng, u["id"])
                            cum[key] = cum.get(key, 0) + u.get(
                                "update_value", 1)
    return json.dumps(d).encode()


def _patched_compile(bir_json, tmpdir, neff_name="file.neff"):
    return _orig_compile(_split_waits(_strip_self_waits(bir_json)),
                         tmpdir, neff_name=neff_name)


def _install_patch():
    bu.compile_bir_kernel = _patched_compile
    b2j.compile_bir_kernel = _patched_compile


F32 = mybir.dt.float32
BF16 = mybir.dt.bfloat16

DIM = 768
NH = 12
HD = 64
SCALE = HD ** -0.5
NB = 2
N = 1024
NT = NB * N
NCC = DIM // 128
NHP = NH // 2
SW = 65  # vp slot width per head ([v | 1])


def build_attention_nc(reps: int = 1):
    nc = bass.Bass("TRN2", target_bir_lowering=False, debug=False)
    xT = nc.declare_dram_parameter("xT", [DIM, NT], BF16, isOutput=False)
    wqkvT = nc.declare_dram_parameter("wqkvT", [DIM, 3 * DIM], BF16, isOutput=False)
    wprojT = nc.declare_dram_parameter("wprojT", [DIM, DIM], BF16, isOutput=False)
    bias = nc.declare_dram_parameter("bias", [DIM, 1], F32, isOutput=False)
    out = nc.declare_dram_parameter("out", [DIM, NT], F32, isOutput=True)

    with tile.TileContext(nc) as tc:
        for rep in range(reps):
            _emit(nc, tc, xT, wqkvT, wprojT, bias, out, rep)
    return nc


def _emit(nc, tc, xT, wqkvT, wprojT, bias, out, rep):
    R = f"r{rep}_"
    with ExitStack() as ctx:
        p_const = ctx.enter_context(tc.tile_pool(name=R + "const", bufs=1))
        p_w = ctx.enter_context(tc.tile_pool(name=R + "w", bufs=1))
        p_qk = ctx.enter_context(tc.tile_pool(name=R + "qk", bufs=1))
        p_vp = ctx.enter_context(tc.tile_pool(name=R + "vp", bufs=1))
        p_aT = ctx.enter_context(tc.tile_pool(name=R + "aT", bufs=1))
        p_e = ctx.enter_context(tc.tile_pool(name=R + "E", bufs=16))
        p_rs = ctx.enter_context(tc.tile_pool(name=R + "rs", bufs=2))
        p_bc = ctx.enter_context(tc.tile_pool(name=R + "bc", bufs=2))
        p_ob = ctx.enter_context(tc.tile_pool(name=R + "ob", bufs=2))

        # ---- constants / weights / inputs ----
        bias_sb = []
        for oc in range(NCC):
            tbs = p_const.tile([128, 1], F32, name=R + f"bias_sb{oc}")
            nc.sync.dma_start(tbs[:], bias[oc * 128:(oc + 1) * 128, :])
            bias_sb.append(tbs)
        ones_bf = p_const.tile([128, 64], BF16, name=R + "ones_bf")
        nc.vector.memset(ones_bf[:], 1.0)

        wp_t = []
        for hp in range(NHP):
            t = p_w.tile([128, DIM], BF16, name=R + f"wp{hp}")
            nc.sync.dma_start(t[:], wprojT[hp * 128:(hp + 1) * 128, :])
            wp_t.append(t)
        if "empty" in _ABLATE:
            with tc.tile_pool(name=R + "pse", bufs=1, space="PSUM") as p_pse, \
                 tc.tile_pool(name=R + "awe", bufs=1) as p_awe:
                xbe = p_awe.tile([128, NT], BF16, name=R + "xbe")
                nc.sync.dma_start(xbe[:], xT[0:128, :])
                pse = p_pse.tile([128, 512], F32, name=R + "pse0")
                nc.tensor.matmul(pse[:], xbe[:, 0:128], xbe[:, 0:512],
                                 start=True, stop=True)
                obe = p_ob.tile([128, N], F32, tag=R + "ob")
                nc.vector.tensor_copy(obe[:, 0:512], pse[:])
                nc.vector.memset(obe[:, 512:1024], 0.0)
                for oc in range(NCC):
                    for b in range(NB):
                        nc.sync.dma_start(
                            out[oc * 128:(oc + 1) * 128, b * N:(b + 1) * N],
                            obe[:])
            return
        qT_t = [p_qk.tile([128, NT], BF16, name=R + f"qT{i}") for i in range(NHP)]
        kT_t = [p_qk.tile([128, NT], BF16, name=R + f"kT{i}") for i in range(NHP)]
        vp_t = [p_vp.tile([128, NH * SW], BF16, name=R + f"vp{i}")
                for i in range(NT // 128)]
        for t in vp_t:  # ones columns for the softmax denominators
            nc.vector.memset(
                t[:].rearrange("p (h e) -> p h e", e=SW)[:, :, HD:SW], 1.0)
        aT_t = {}
        e_static = None
        if "noexp" in _ABLATE:
            e_static = p_const.tile([128, 512], BF16, name=R + "e_static")
            nc.vector.memset(e_static[:], 0.002)

        # ---- phase A: projections (each stationary streams 2 matmuls) ----
        if "onlyB" in _ABLATE:
            for t in qT_t + kT_t:
                nc.vector.memset(t[:], 0.01)
            for t in vp_t:
                nc.vector.memset(
                    t[:].rearrange("p (h e) -> p h e", e=SW)[:, :, 0:HD], 0.01)
        if "onlyB" not in _ABLATE:
          with tc.tile_pool(name=R + "psA", bufs=2, space="PSUM") as p_psA, \
             tc.tile_pool(name=R + "aw", bufs=1) as p_aw:
            wq_t = []
            for c in range(NCC):
                t = p_aw.tile([128, 3 * DIM], BF16, name=R + f"wq{c}")
                nc.sync.dma_start(t[:], wqkvT[c * 128:(c + 1) * 128, :])
                wq_t.append(t)
            xb = []
            for c in range(NCC):
                t = p_aw.tile([128, NT], BF16, name=R + f"xb{c}")
                nc.sync.dma_start(t[:], xT[c * 128:(c + 1) * 128, :])
                xb.append(t)
            for nn in range(NT // 128):
                ps = p_psA.tile([128, 1024], F32, tag=R + "psA",
                                name=R + f"psv{nn}")
                for c in range(NCC):
                    nc.tensor.matmul(
                        ps[:, 0:512],
                        xb[c][:, nn * 128:(nn + 1) * 128],
                        wq_t[c][:, 2 * DIM:2 * DIM + 512],
                        start=(c == 0), stop=(c == NCC - 1),
                    )
                    nc.tensor.matmul(
                        ps[:, 512:768],
                        xb[c][:, nn * 128:(nn + 1) * 128],
                        wq_t[c][:, 2 * DIM + 512:3 * DIM],
                        start=(c == 0), stop=(c == NCC - 1),
                    )
                nc.vector.tensor_copy(
                    vp_t[nn][:].rearrange("p (h e) -> p h e", e=SW)[:, :, 0:HD],
                    ps[:, 0:768].rearrange("p (h d) -> p h d", d=HD),
                )
            for hp in range(NHP):
                for which, dst in ((0, qT_t[hp]), (1, kT_t[hp])):
                    wcol = (which * NHP + hp) * 128
                    for half in range(2):
                        ps = p_psA.tile([128, 1024], F32, tag=R + "psA",
                                        name=R + f"psq{hp}_{which}_{half}")
                        for c in range(NCC):
                            for q in range(2):
                                nc.tensor.matmul(
                                    ps[:, q * 512:(q + 1) * 512],
                                    wq_t[c][:, wcol:wcol + 128],
                                    xb[c][:, half * 1024 + q * 512:
                                          half * 1024 + (q + 1) * 512],
                                    start=(c == 0), stop=(c == NCC - 1),
                                )
                        nc.vector.tensor_copy(
                            dst[:, half * 1024:(half + 1) * 1024], ps[:])

        # ---- phase B: attention, software-pipelined with staged epilogues ----
        LAG = 2
        with tc.tile_pool(name=R + "psS", bufs=4, space="PSUM") as p_psS, \
             tc.tile_pool(name=R + "pa", bufs=4, space="PSUM") as p_pa:

            def make_stages(at, pa, e_q, hp, b, boff):
                """Deferred work of this block, one stage per step of the
                NEXT block: lagged AVs; denominator+output copies (free pa);
                partition-gather of the 4 sum rows; ONE wide reciprocal;
                bf16 broadcasts; in-place normalize."""
                drow = p_rs.tile([128, 2048], BF16, tag=R + "drow",
                                 name=R + f"drow{b}_{hp}")
                st = {}
                P_OF = {0: 0, 1: 32, 2: 64, 3: 96}

                def do_av(mc):
                    first, last = (mc == 0), (mc == N // 128 - 1)
                    for hi in range(2):
                        vslot = vp_t[(boff + mc * 128) // 128][
                            :, (2 * hp + hi) * SW:(2 * hp + hi + 1) * SW]
                        for nh in range(2):
                            esrc = e_q.pop((mc, hi, nh))
                            if "noexp" in _ABLATE:
                                esrc = e_static
                            nc.tensor.matmul(
                                pa[2 * hi + nh][0:SW, :],
                                vslot,
                                esrc[:],
                                start=first, stop=last,
                            )

                def s0():
                    do_av(N // 128 - 2)

                def s1():
                    do_av(N // 128 - 1)
                    for j in range(4):  # denominator rows -> drow (bf16)
                        nc.vector.tensor_copy(
                            drow[64:65, j * 512:(j + 1) * 512],
                            pa[j][64:65, :])
                    for j in (0, 1):
                        nc.vector.tensor_copy(
                            at[0:64, (j % 2) * 512:(j % 2 + 1) * 512],
                            pa[j][0:64, :])

                def s2():
                    for j in (2, 3):
                        nc.vector.tensor_copy(
                            at[64:128, (j % 2) * 512:(j % 2 + 1) * 512],
                            pa[j][0:64, :])
                    psG = p_psS.tile([128, 512], F32, tag=R + "psS",
                                     name=R + f"psG{b}_{hp}")
                    nc.vector.memset(psG[:], 1.0)
                    for j in range(4):  # gather sums to partitions 0/32/64/96
                        nc.tensor.matmul(
                            psG[P_OF[j]:P_OF[j] + 1, :],
                            ones_bf[64:65, 0:1],
                            drow[64:65, j * 512:(j + 1) * 512],
                            start=True, stop=True,
                            tile_position=(64, P_OF[j]),
                            skip_group_check=True,
                        )
                    st["psG"] = psG

                def s3():
                    rs2 = p_rs.tile([128, 512], F32, tag=R + "rs2",
                                    name=R + f"rs2{b}_{hp}")
                    nc.vector.reciprocal(rs2[:], st["psG"][:])
                    rsb = p_rs.tile([128, 512], BF16, tag=R + "rsb",
                                    name=R + f"rsb{b}_{hp}")
                    nc.vector.tensor_copy(rsb[:], rs2[:])
                    st["rsb"] = rsb

                def s4():
                    for nh in range(2):
                        psB = p_psS.tile([128, 512], F32, tag=R + "psS",
                                         name=R + f"psB{b}_{hp}_{nh}")
                        for hi in range(2):
                            p = P_OF[2 * hi + nh]
                            nc.tensor.matmul(
                                psB[hi * 64:(hi + 1) * 64, :],
                                ones_bf[p:p + 1, 0:64],
                                st["rsb"][p:p + 1, :],
                                start=True, stop=True,
                                tile_position=(p, hi * 64),
                                skip_group_check=True,
                            )
                        bc = p_bc.tile([128, 512], BF16, tag=R + "bc")
                        nc.vector.tensor_copy(bc[:], psB[:])
                        st[f"bc{nh}"] = bc

                def s5():
                    for nh in range(2):
                        nc.vector.tensor_mul(
                            at[:, nh * 512:(nh + 1) * 512],
                            at[:, nh * 512:(nh + 1) * 512],
                            st[f"bc{nh}"][:])

                if "noepi" in _ABLATE:
                    def s1b():
                        do_av(N // 128 - 1)
                        for j in range(4):
                            nc.vector.tensor_copy(
                                at[(j // 2) * 64:(j // 2 + 1) * 64,
                                   (j % 2) * 512:(j % 2 + 1) * 512],
                                pa[j][0:64, :])
                    return [s0, s1b]
                return [s0, s1, s2, s3, s4, s5]

            stages = []
            if "noB" in _ABLATE:
                for hp in range(NHP):
                    for b in range(NB):
                        at = p_aT.tile([128, N], BF16, name=R + f"aT{b}_{hp}")
                        nc.vector.memset(at[:], 0.01)
                        aT_t[(b, hp)] = at
            for hp in range(NHP if "noB" not in _ABLATE else 0):
                for b in range(NB):
                    boff = b * N
                    at = p_aT.tile([128, N], BF16, name=R + f"aT{b}_{hp}")
                    e_q = {}
                    pa = None
                    for mc in range(N // 128):
                        for hi in range(2):
                            pb_ = hi * 64
                            pss = []
                            for nh in range(2):
                                ps = p_psS.tile(
                                    [128, 512], F32, tag=R + "psS",
                                    name=R + f"psS{b}_{hp}_{mc}_{hi}_{nh}")
                                nc.tensor.matmul(
                                    ps[:],
                                    kT_t[hp][pb_:pb_ + 64,
                                             boff + mc * 128:
                                             boff + (mc + 1) * 128],
                                    qT_t[hp][pb_:pb_ + 64,
                                             boff + nh * 512:
                                             boff + (nh + 1) * 512],
                                    start=True, stop=True,
                                    tile_position=(pb_, 0),
                                )
                                pss.append(ps)
                            for nh in range(2):
                                e = p_e.tile([128, 512], BF16, tag=R + "E")
                                nc.scalar.activation(
                                    e[:], pss[nh][:],
                                    mybir.ActivationFunctionType.Exp,
                                    scale=SCALE,
                                )
                                e_q[(mc, hi, nh)] = e
                        if stages:
                            stages.pop(0)()
                        if mc == LAG:
                            pa = [p_pa.tile([128, 512], F32, tag=R + "pa",
                                            name=R + f"pa{b}_{hp}_{j}")
                                  for j in range(4)]
                        if mc >= LAG:
                            first = (mc - LAG == 0)
                            for hi in range(2):
                                vslot = vp_t[(boff + (mc - LAG) * 128) // 128][
                                    :, (2 * hp + hi) * SW:
                                    (2 * hp + hi + 1) * SW]
                                for nh in range(2):
                                    esrc = e_q.pop((mc - LAG, hi, nh))
                                    if "noexp" in _ABLATE:
                                        esrc = e_static
                                    nc.tensor.matmul(
                                        pa[2 * hi + nh][0:SW, :],
                                        vslot,
                                        esrc[:],
                                        start=first, stop=False,
                                    )
                    aT_t[(b, hp)] = at
                    stages = make_stages(at, pa, e_q, hp, b, boff)
            for s in stages:
                s()

            # ---- phase C: transposed output projection ----
            for oc in range(NCC):
                psC = [p_psS.tile([128, 512], F32, tag=R + "psS",
                                  name=R + f"psC{oc}_{u}") for u in range(4)]
                for cp in range(NHP):
                    lhs = wp_t[cp][:, oc * 128:(oc + 1) * 128]
                    for b in range(NB):
                        for nh in range(2):
                            nc.tensor.matmul(
                                psC[2 * b + nh][:],
                                lhs,
                                aT_t[(b, cp)][:, nh * 512:(nh + 1) * 512],
                                start=(cp == 0), stop=(cp == NHP - 1))
                for b in range(NB):
                    ob = p_ob.tile([128, N], F32, tag=R + "ob")
                    for nh in range(2):
                        nc.vector.tensor_scalar_add(
                            ob[:, nh * 512:(nh + 1) * 512],
                            psC[2 * b + nh][:], bias_sb[oc][:])
                    nc.sync.dma_start(
                        out[oc * 128:(oc + 1) * 128, b * N:(b + 1) * N], ob[:])


# ---------------------------------------------------------------------------
# host wrapper
# ---------------------------------------------------------------------------
_CACHE = {}


def _prep_in_maps(x, w_qkv, w_proj, b_proj):
    x = np.asarray(x, dtype=np.float32)
    wqkvT = np.ascontiguousarray(np.asarray(w_qkv, dtype=np.float32).T
                                 ).astype(ml_dtypes.bfloat16)
    wprojT = np.ascontiguousarray(np.asarray(w_proj, dtype=np.float32).T
                                  ).astype(ml_dtypes.bfloat16)
    bias = np.asarray(b_proj, dtype=np.float32).reshape(DIM, 1).copy()
    in_maps = []
    for c in range(N_CORES):
        xs = x[c * NB:(c + 1) * NB]                       # [2, 1024, 768]
        xT = np.ascontiguousarray(xs.transpose(2, 0, 1).reshape(DIM, NT))
        in_maps.append({
            "xT": xT.astype(ml_dtypes.bfloat16),
            "wqkvT": wqkvT,
            "wprojT": wprojT,
            "bias": bias,
        })
    return in_maps


def kernel(x, w_qkv, w_proj, b_proj):
    _install_patch()
    if "nc" not in _CACHE:
        _CACHE["nc"] = build_attention_nc(1)
    nc = _CACHE["nc"]
    in_maps = _prep_in_maps(x, w_qkv, w_proj, b_proj)
    res = run_bass_kernel_spmd(nc, in_maps, core_ids=list(range(N_CORES)))
    shards = []
    for c in range(N_CORES):
        oT = res.results[c]["out"]                        # [768, 2048]
        shards.append(oT.T.reshape(NB, N, DIM))
    return np.ascontiguousarray(
        np.concatenate(shards, axis=0)).astype(np.float32)
